# revision 4
# baseline (speedup 1.0000x reference)
"""Trainium2 Bass kernel for the AttentionBlock problem.

Reference semantics (shapes hardcoded):
    x [4, 256, 64, 64]; 1x1-conv weights q_w/k_w/v_w [256, 258] (+biases),
    fc_w [256, 256], fc_b [256].
    x0 = concat(x, pos) -> [B, 258, 4096]
    q/k/v = relu(W @ x0 + b)                    [B, 256, 4096]
    attn  = softmax_causal(q^T k)               [B, 4096, 4096]
    out   = x + relu(fc_w @ (attn @ v^T)^T + fc_b)

Distribution: 8 cores = 4 batches x 2 query-block roles. Each core
computes full k / v^T for its batch, q only for its 4 owned 512-wide
query blocks, and causal attention for those blocks. Causal work is
balanced by giving role 0 global blocks [0,3,4,7] and role 1 blocks
[1,2,5,6]; both roles run the identical SPMD program with per-slot
key-tile counts [8,16,24,32] (slightly padded); per-core mask data
zeroes padded/non-causal entries.

Precision plan (tolerance 2e-2; host-simulated error ~1.2e-2):
  - projections + scores in bf16 (PSUM f32).  The pos+bias part of
    each projection is a batch-independent map precomputed on the
    host, DMAed as bf16, added to the PSUM on VectorE; relu on GpSimd.
  - softmax without max-subtraction: p = exp(s) in bf16 (scores are
    ~20..67, far below overflow), den1 = ones-matmul of GpSimd quad
    sums, with the ones matrix scaled by e^-5.
  - attn@v in fp8e4m3 DoubleRow (2 key tiles per PE instruction):
    p_hat = fp8(p * F), F = recip(e^-5 * den1).  A per-query factor
    cancels in softmax normalization, so F only needs to put p_hat in
    fp8 range, which it provably does: p_hat <= e^5 = 148 < 240
    (fp8e4m3 max) and the top weight is >= e^5/4096 (normal range).
    The final normalizer den2 re-sums the actual fp8 p_hat values
    (fp8-ones DoubleRow matmuls) so fp8 rounding mostly cancels
    between numerator and denominator.
  - fc + residual in bf16/f32.

Engine budget: PE does only bf16/fp8 matmuls (no f32r anywhere, 2
dtype switches per slot); ScalarE: exp + fc relu + F copies; VectorE:
pos adds, rescale muls, reciprocals, normalize/residual; GpSimd:
relus, mask muls, den1 quad sums.
"""

import numpy as np

B = 4
C = 256
S = 64
N = S * S            # 4096
K = 256              # q/k/v channels
NBLK = 512           # query block width
NSLOT = 4            # owned query blocks per core
M_S = (8, 16, 24, 32)  # key-tile count per slot (128-wide key tiles)
BLOCKS = ((0, 3, 4, 7), (1, 2, 5, 6))  # role -> global block ids

_PROGRAM = None


def _build_program():
    import concourse.bacc as bacc
    import concourse.mybir as mybir
    import concourse.tile as tile

    F32 = mybir.dt.float32
    BF16 = mybir.dt.bfloat16
    FP8 = mybir.dt.float8e4
    DR = mybir.MatmulPerfMode.DoubleRow
    Act = mybir.ActivationFunctionType

    nc = bacc.Bacc("TRN2", target_bir_lowering=False, debug=False)

    x0b_d = nc.dram_tensor("x0b", [C, N], BF16, kind="ExternalInput")
    xq_d = nc.dram_tensor("xq", [C, NSLOT * NBLK], BF16, kind="ExternalInput")
    pmk_d = nc.dram_tensor("pmk", [K, N], BF16, kind="ExternalInput")
    pmq_d = nc.dram_tensor("pmq", [K, NSLOT * NBLK], BF16,
                           kind="ExternalInput")
    pmv_d = nc.dram_tensor("pmv", [N, K], BF16, kind="ExternalInput")
    wq_d = nc.dram_tensor("wq", [C, K], BF16, kind="ExternalInput")
    wk_d = nc.dram_tensor("wk", [C, K], BF16, kind="ExternalInput")
    wv_d = nc.dram_tensor("wv", [C, K], BF16, kind="ExternalInput")
    fcw_d = nc.dram_tensor("fcw", [C, C], BF16, kind="ExternalInput")
    fcb_d = nc.dram_tensor("fcb", [C, 1], F32, kind="ExternalInput")
    msk_d = nc.dram_tensor("masks", [NSLOT, 8, 128, NBLK], BF16,
                           kind="ExternalInput")
    od_d = nc.dram_tensor("onesd", [128, 128], BF16, kind="ExternalInput")
    o8_d = nc.dram_tensor("ones8", [128, 256], FP8, kind="ExternalInput")
    xres_d = nc.dram_tensor("xres", [C, NSLOT * NBLK], F32,
                            kind="ExternalInput")
    out_d = nc.dram_tensor("out", [C, NSLOT * NBLK], F32, kind="ExternalOutput")

    with tile.TileContext(nc) as tc:
        with (
            tc.tile_pool(name="wts", bufs=1) as wts,
            tc.tile_pool(name="pmv_p", bufs=9) as pmv_p,
            tc.tile_pool(name="tmp_p", bufs=4) as tmp_p,
            tc.tile_pool(name="kqv_p", bufs=1) as kqv_p,
            tc.tile_pool(name="msk_p", bufs=6) as msk_p,
            tc.tile_pool(name="ex_p", bufs=36) as ex_p,
            tc.tile_pool(name="ep_p", bufs=18) as ep_p,
            tc.tile_pool(name="ds_p", bufs=4) as ds_p,
            tc.tile_pool(name="f_p", bufs=2) as f_p,
            tc.tile_pool(name="o_p", bufs=2) as o_p,
            tc.tile_pool(name="tr_p", bufs=1) as tr_p,
            tc.tile_pool(name="ps_sc", bufs=3, space="PSUM") as ps_sc,
            tc.tile_pool(name="ps_d1", bufs=1, space="PSUM") as ps_d1,
            tc.tile_pool(name="ps_o", bufs=1, space="PSUM") as ps_o,
            tc.tile_pool(name="ps_d2", bufs=1, space="PSUM") as ps_d2,
            tc.tile_pool(name="ps_fc", bufs=1, space="PSUM") as ps_fc,
        ):
            def wtile(dram, r0, rn, dt, tag, shape=None):
                t = wts.tile(shape or [rn, dram.shape[1]], dt, tag=tag,
                             name=tag)
                nc.sync.dma_start(t[:], dram[r0:r0 + rn, :])
                return t

            # phase-A-first weights (k, v) so PE can start early
            wk_t = [wtile(wk_d, 0, 128, BF16, "wk0"),
                    wtile(wk_d, 128, 128, BF16, "wk1")]
            wv_t = [wtile(wv_d, 0, 128, BF16, "wv0"),
                    wtile(wv_d, 128, 128, BF16, "wv1")]

            # resident x0b / pmk, DMAed in per-pair column chunks so the
            # first projections are not gated on the full transfer
            x0_t = [kqv_p.tile([128, N], BF16, tag=f"x0{ci}", name=f"x0{ci}")
                    for ci in range(2)]
            pmk_t = [kqv_p.tile([128, N], BF16, tag=f"pmk{kt}",
                                name=f"pmk{kt}") for kt in range(2)]

            def chunk_dma(nbp):
                sl = slice(1024 * nbp, 1024 * (nbp + 1))
                for ci in range(2):
                    nc.sync.dma_start(x0_t[ci][:, sl],
                                      x0b_d[128 * ci:128 * (ci + 1), sl])
                for kt in range(2):
                    nc.sync.dma_start(pmk_t[kt][:, sl],
                                      pmk_d[128 * kt:128 * (kt + 1), sl])

            chunk_dma(0)

            k_sb = [[None] * 8 for _ in range(2)]
            vpair = [None] * 16  # fp8 [128, 2, 256] per key-tile pair

            def emit_pair(nbp):
                for nb in (2 * nbp, 2 * nbp + 1):
                    sl = slice(NBLK * nb, NBLK * (nb + 1))
                    for kt in range(2):
                        kts = slice(128 * kt, 128 * (kt + 1))
                        pk = ps_sc.tile([128, NBLK], F32, tag="sc",
                                        name=f"pk{kt}_{nb}")
                        nc.tensor.matmul(pk[:], wk_t[0][:, kts],
                                         x0_t[0][:, sl], start=True,
                                         stop=False)
                        nc.tensor.matmul(pk[:], wk_t[1][:, kts],
                                         x0_t[1][:, sl], start=False,
                                         stop=True)
                        kt_tmp = tmp_p.tile([128, NBLK], BF16, tag="ktmp",
                                            name=f"ktmp{kt}_{nb}")
                        nc.vector.tensor_add(kt_tmp[:], pk[:],
                                             pmk_t[kt][:, sl])
                        kt_sb = kqv_p.tile([128, NBLK], BF16,
                                           tag=f"k{kt}_{nb}",
                                           name=f"k{kt}_{nb}")
                        nc.gpsimd.tensor_relu(kt_sb[:], kt_tmp[:])
                        k_sb[kt][nb] = kt_sb
                for nb in (2 * nbp, 2 * nbp + 1):
                    for sub in range(4):
                        i = 4 * nb + sub
                        ss = slice(128 * i, 128 * (i + 1))
                        pmv_t = pmv_p.tile([128, K], BF16, tag="pmv",
                                           name=f"pmv{i}")
                        nc.sync.dma_start(pmv_t[:], pmv_d[ss, :])
                        pv = ps_sc.tile([128, K], F32, tag="sc",
                                        name=f"pv{i}")
                        nc.tensor.matmul(pv[:], x0_t[0][:, ss], wv_t[0][:],
                                         start=True, stop=False)
                        nc.tensor.matmul(pv[:], x0_t[1][:, ss], wv_t[1][:],
                                         start=False, stop=True)
                        vt_tmp = tmp_p.tile([128, K], BF16, tag="vtmp",
                                            name=f"vtmp{i}")
                        nc.vector.tensor_add(vt_tmp[:], pv[:], pmv_t[:])
                        if vpair[i // 2] is None:
                            vpair[i // 2] = kqv_p.tile(
                                [128, 2, K], FP8, tag=f"vp{i // 2}",
                                name=f"vp{i // 2}")
                        nc.gpsimd.tensor_relu(
                            vpair[i // 2][:, i % 2:i % 2 + 1, :], vt_tmp[:])

            q_sb = [[None] * NSLOT for _ in range(2)]

            def emit_q(s):
                sl = slice(NBLK * s, NBLK * (s + 1))
                for kt in range(2):
                    kts = slice(128 * kt, 128 * (kt + 1))
                    pq = ps_sc.tile([128, NBLK], F32, tag="sc",
                                    name=f"pq{kt}_{s}")
                    nc.tensor.matmul(pq[:], wq_t[0][:, kts], xq_t[0][:, sl],
                                     start=True, stop=False)
                    nc.tensor.matmul(pq[:], wq_t[1][:, kts], xq_t[1][:, sl],
                                     start=False, stop=True)
                    q_tmp = tmp_p.tile([128, NBLK], BF16, tag="qtmp",
                                       name=f"qtmp{kt}_{s}")
                    nc.vector.tensor_add(q_tmp[:], pq[:], pmq_t[kt][:, sl])
                    qt = kqv_p.tile([128, NBLK], BF16, tag=f"q{kt}_{s}",
                                    name=f"q{kt}_{s}")
                    nc.gpsimd.tensor_relu(qt[:], q_tmp[:])
                    q_sb[kt][s] = qt

            slot_state = {}  # s -> [epairs, po, pd2]

            def emit_scores(s):
                """scores + exp + masks + den1 + F + rescale for slot s."""
                M = M_S[s]
                pd1 = ps_d1.tile([128, NBLK], F32, tag="d1", name=f"pd1{s}")
                ex_tiles = [None] * M
                for i in range(M):
                    psc = ps_sc.tile([128, NBLK], F32, tag="sc",
                                     name=f"psc{s}_{i}")
                    for kt in range(2):
                        nc.tensor.matmul(
                            psc[:],
                            k_sb[kt][i // 4][:, 128 * (i % 4):128 * (i % 4 + 1)],
                            q_sb[kt][s][:], start=(kt == 0), stop=(kt == 1))
                    ex = ex_p.tile([128, NBLK], BF16, tag="ex",
                                   name=f"ex{s}_{i}")
                    nc.scalar.activation(ex[:], psc[:], Act.Exp)
                    if i >= M - 8:
                        mk = msk_p.tile([128, NBLK], BF16, tag="mk",
                                        name=f"mk{s}_{i}")
                        nc.sync.dma_start(mk[:], msk_d[s, i - (M - 8)])
                        nc.gpsimd.tensor_mul(ex[:], ex[:], mk[:])
                    ex_tiles[i] = ex
                    if i % 4 == 3:
                        j = i - 3
                        da = ds_p.tile([128, NBLK], BF16, tag="ds",
                                       name=f"da{s}_{j}")
                        nc.gpsimd.tensor_add(da[:], ex_tiles[j][:],
                                             ex_tiles[j + 1][:])
                        db = ds_p.tile([128, NBLK], BF16, tag="ds",
                                       name=f"db{s}_{j}")
                        nc.gpsimd.tensor_add(db[:], ex_tiles[j + 2][:],
                                             ex_tiles[j + 3][:])
                        dsum = ds_p.tile([128, NBLK], BF16, tag="ds",
                                         name=f"ds{s}_{j}")
                        nc.gpsimd.tensor_add(dsum[:], da[:], db[:])
                        nc.tensor.matmul(pd1[:], od_t[:], dsum[:],
                                         start=(j == 0), stop=(j == M - 4))
                # F = recip(e^-5 * den1); bf16 copy for fast rescale muls
                f32_t = f_p.tile([128, NBLK], F32, tag="f32", name=f"f32{s}")
                nc.vector.reciprocal_approx_fast(f32_t[:], pd1[:])
                fb_t = f_p.tile([128, NBLK], BF16, tag="fb", name=f"fb{s}")
                nc.scalar.copy(fb_t[:], f32_t[:])
                epairs = []
                for p in range(M // 2):
                    ep = ep_p.tile([128, 2, NBLK], FP8, tag="ep",
                                   name=f"ep{s}_{p}")
                    for h in range(2):
                        nc.vector.tensor_mul(ep[:, h:h + 1, :],
                                             ex_tiles[2 * p + h][:], fb_t[:])
                    epairs.append(ep)
                slot_state[s] = [epairs, None, None]

            def emit_av(s):
                """fp8 DoubleRow attn@v + den2 for slot s."""
                M = M_S[s]
                epairs = slot_state[s][0]
                po = [ps_o.tile([128, NBLK], F32, tag=f"o{vt}",
                                name=f"po{vt}_{s}") for vt in range(2)]
                pd2 = ps_d2.tile([128, NBLK], F32, tag="d2", name=f"pd2{s}")
                np_ = M // 2
                for p in range(np_):
                    vp = vpair[p]
                    ep = epairs[p]
                    for vt in range(2):
                        nc.tensor.matmul(
                            po[vt][:], vp[:, :, 128 * vt:128 * (vt + 1)],
                            ep[:], start=(p == 0), stop=(p == np_ - 1),
                            perf_mode=DR)
                    nc.tensor.matmul(pd2[:], o8_t[:], ep[:],
                                     start=(p == 0), stop=(p == np_ - 1),
                                     perf_mode=DR)
                slot_state[s][1] = po
                slot_state[s][2] = pd2

            def finalize(s):
                """normalize, fc, relu, residual, dma out for slot s."""
                epairs, po, pd2 = slot_state[s]
                rb2 = f_p.tile([128, NBLK], F32, tag="rb2", name=f"rb2{s}")
                nc.vector.reciprocal_approx_fast(rb2[:], pd2[:])
                o_sb = []
                for vt in range(2):
                    ot = o_p.tile([128, NBLK], BF16, tag=f"ob{vt}",
                                  name=f"ob{vt}_{s}")
                    nc.vector.tensor_mul(ot[:], po[vt][:], rb2[:])
                    o_sb.append(ot)
                for ot in range(2):
                    pfc = ps_fc.tile([128, NBLK], F32, tag="fc",
                                     name=f"pfc{ot}_{s}")
                    for vt in range(2):
                        nc.tensor.matmul(
                            pfc[:], fcw_t[vt][:, 128 * ot:128 * (ot + 1)],
                            o_sb[vt][:], start=(vt == 0), stop=(vt == 1))
                    t_sb = tr_p.tile([128, NBLK], F32, tag=f"t{ot}",
                                     name=f"t{ot}_{s}")
                    nc.scalar.activation(t_sb[:], pfc[:], Act.Relu,
                                         bias=fcb_t[ot][:])
                    r_sb = tr_p.tile([128, NBLK], F32, tag=f"r{ot}",
                                     name=f"r{ot}_{s}")
                    nc.vector.tensor_add(
                        r_sb[:], t_sb[:],
                        xres_t[ot][:, NBLK * s:NBLK * (s + 1)])
                    nc.sync.dma_start(
                        out_d[128 * ot:128 * (ot + 1),
                              NBLK * s:NBLK * (s + 1)], r_sb[:])

            # ---- emission schedule ----
            # PE stream: P0 Q0 S0 | P1 Q1 | A0 | S1 P2 Q2 | A1 | F0 S2 P3
            # Q3 | A2 | F1 S3 | A3 | F2 F3 -- av batches (fp8) are covered
            # by the next slot's bf16 work so the PE never waits on the
            # DVE rescale; 2 dtype switches per slot.
            emit_pair(0)

            wq_t = [wtile(wq_d, 0, 128, BF16, "wq0"),
                    wtile(wq_d, 128, 128, BF16, "wq1")]
            xq_t = [wtile(xq_d, 0, 128, BF16, "xq0"),
                    wtile(xq_d, 128, 128, BF16, "xq1")]
            pmq_t = [wtile(pmq_d, 128 * kt, 128, BF16, f"pmq{kt}")
                     for kt in range(2)]
            fcw_t = [wtile(fcw_d, 0, 128, BF16, "fcw0"),
                     wtile(fcw_d, 128, 128, BF16, "fcw1")]
            fcb_t = [wtile(fcb_d, 0, 128, F32, "fcb0"),
                     wtile(fcb_d, 128, 128, F32, "fcb1")]
            od_t = wtile(od_d, 0, 128, BF16, "onesd")
            o8_t = wtile(o8_d, 0, 128, FP8, "ones8", shape=[128, 2, 128])
            xres_t = [wtile(xres_d, 128 * ot, 128, F32, f"xres{ot}")
                      for ot in range(2)]

            emit_q(0)
            emit_scores(0)
            chunk_dma(1)
            emit_pair(1)
            emit_q(1)
            emit_av(0)
            emit_scores(1)
            chunk_dma(2)
            emit_pair(2)
            emit_q(2)
            emit_av(1)
            finalize(0)
            emit_scores(2)
            chunk_dma(3)
            emit_pair(3)
            emit_q(3)
            emit_av(2)
            finalize(1)
            emit_scores(3)
            emit_av(3)
            finalize(2)
            finalize(3)

    nc.compile()
    return nc


def _host_prep(x, q_w, q_b, k_w, k_b, v_w, v_b, fc_w, fc_b):
    """Build the per-core input maps."""
    import ml_dtypes
    f32 = np.float32
    bf16 = ml_dtypes.bfloat16
    fp8 = ml_dtypes.float8_e4m3
    n = np.arange(N)
    px = ((n // S) / S).astype(f32)
    py = ((n % S) / S).astype(f32)
    pos = np.stack([px, py])                      # [2, N]

    # batch-independent pos+bias maps of each projection
    pm_q = (q_w.astype(f32)[:, C:] @ pos + q_b.astype(f32)[:, None])
    pm_k = (k_w.astype(f32)[:, C:] @ pos + k_b.astype(f32)[:, None])
    pm_v = (v_w.astype(f32)[:, C:] @ pos + v_b.astype(f32)[:, None])

    # per-role masks [NSLOT, 8, 128, 512]
    mm = np.arange(128)[:, None]
    nn = np.arange(NBLK)[None, :]
    masks = {}
    for r in range(2):
        mr = np.zeros((NSLOT, 8, 128, NBLK), f32)
        for s in range(NSLOT):
            j = BLOCKS[r][s]
            for t in range(8):
                i = M_S[s] - 8 + t
                mr[s, t] = (128 * i + mm <= 512 * j + nn)
        masks[r] = mr.astype(bf16)

    shared = {
        "wq": np.ascontiguousarray(q_w.astype(f32).T[:C]).astype(bf16),
        "wk": np.ascontiguousarray(k_w.astype(f32).T[:C]).astype(bf16),
        "wv": np.ascontiguousarray(v_w.astype(f32).T[:C]).astype(bf16),
        "pmk": pm_k.astype(bf16),
        "pmv": np.ascontiguousarray(pm_v.T).astype(bf16),
        "fcw": np.ascontiguousarray(fc_w.astype(f32).T).astype(bf16),
        "fcb": np.ascontiguousarray(fc_b.astype(f32).reshape(C, 1)),
        "onesd": np.full((128, 128), np.exp(-5.0), f32).astype(bf16),
        "ones8": np.ones((128, 256), f32).astype(fp8),
    }

    in_maps = []
    for c in range(8):
        b, r = c // 2, c % 2
        xb = x[b].reshape(C, N).astype(f32)
        qcols = np.concatenate(
            [np.arange(NBLK * j, NBLK * (j + 1)) for j in BLOCKS[r]])
        in_maps.append(dict(
            shared,
            x0b=xb.astype(bf16),
            xq=np.ascontiguousarray(xb[:, qcols]).astype(bf16),
            xres=np.ascontiguousarray(xb[:, qcols]),
            pmq=np.ascontiguousarray(pm_q[:, qcols]).astype(bf16),
            masks=masks[r],
        ))
    return in_maps


def _gather(results):
    out = np.empty((B, C, N), np.float32)
    for c in range(8):
        b, r = c // 2, c % 2
        oc = results[c]["out"]
        for s, j in enumerate(BLOCKS[r]):
            out[b][:, NBLK * j:NBLK * (j + 1)] = oc[:, NBLK * s:NBLK * (s + 1)]
    return out.reshape(B, C, S, S)


def run(trace=False, **inputs):
    from concourse import bass_utils
    global _PROGRAM
    if _PROGRAM is None:
        _PROGRAM = _build_program()
    in_maps = _host_prep(**inputs)
    res = bass_utils.run_bass_kernel_spmd(
        _PROGRAM, in_maps, list(range(8)), trace=trace)
    return _gather(res.results), res


def kernel(**inputs):
    out, _ = run(trace=False, **inputs)
    return out


# revision 5
# speedup vs baseline: 3.2406x; 3.2406x over previous
"""Trainium2 Bass kernel for the AttentionBlock problem.

Reference semantics (shapes hardcoded):
    x [4, 256, 64, 64]; 1x1-conv weights q_w/k_w/v_w [256, 258] (+biases),
    fc_w [256, 256], fc_b [256].
    x0 = concat(x, pos) -> [B, 258, 4096]
    q/k/v = relu(W @ x0 + b)                    [B, 256, 4096]
    attn  = softmax_causal(q^T k)               [B, 4096, 4096]
    out   = x + relu(fc_w @ (attn @ v^T)^T + fc_b)

Distribution: 8 cores = 4 batches x 2 query-block roles. Each core
computes full k / v^T for its batch, q only for its 4 owned 512-wide
query blocks, and causal attention for those blocks. Causal work is
balanced by giving role 0 global blocks [0,3,4,7] and role 1 blocks
[1,2,5,6]; both roles run the identical SPMD program with per-slot
key-tile counts [8,16,24,32] (slightly padded); per-core mask data
zeroes padded/non-causal entries.

The whole kernel runs in bf16 (PSUM f32): measured on this hardware,
bf16 and f32r matmuls issue at the same rate, but f32r weight loads
are 2x slower and f32r<->bf16 switches cost ~250-450ns, so all-bf16
both removes every switch and halves weight-load time.  Host-sim
error of the all-bf16 path is ~0.9e-2 vs the 2e-2 tolerance.
Softmax runs without max-subtraction (scores ~20..67, far below
overflow); denominator via a replicated ones-matmul of quad sums.
Engine placement (measured: GpSimd is pathologically slow - avoid):
ScalarE exp+relus, VectorE masks/quads/normalize/residual, PE all
matmuls including the rank-3 pos+bias accumulation terms.
"""

import numpy as np

B = 4
C = 256
S = 64
N = S * S            # 4096
K = 256              # q/k/v channels
NBLK = 512           # query block width
NSLOT = 4            # owned query blocks per core
M_S = (8, 16, 24, 32)  # key-tile count per slot (128-wide key tiles)
BLOCKS = ((0, 3, 4, 7), (1, 2, 5, 6))  # role -> global block ids

_PROGRAM = None


def _build_program():
    import concourse.bacc as bacc
    import concourse.mybir as mybir
    import concourse.tile as tile

    F32 = mybir.dt.float32
    BF16 = mybir.dt.bfloat16
    Act = mybir.ActivationFunctionType

    nc = bacc.Bacc("TRN2", target_bir_lowering=False, debug=False)

    x0b_d = nc.dram_tensor("x0b", [C, N], BF16, kind="ExternalInput")
    x0p_d = nc.dram_tensor("x0p", [128, N], BF16, kind="ExternalInput")
    x0c_d = nc.dram_tensor("x0c", [3, N], BF16, kind="ExternalInput")
    xq_d = nc.dram_tensor("xq", [C, NSLOT * NBLK], BF16, kind="ExternalInput")
    xqp_d = nc.dram_tensor("xqp", [128, NSLOT * NBLK], BF16,
                           kind="ExternalInput")
    wq_d = nc.dram_tensor("wq", [C, K], BF16, kind="ExternalInput")
    wk_d = nc.dram_tensor("wk", [C, K], BF16, kind="ExternalInput")
    wv_d = nc.dram_tensor("wv", [C, K], BF16, kind="ExternalInput")
    wqp_d = nc.dram_tensor("wqp", [128, K], BF16, kind="ExternalInput")
    wkp_d = nc.dram_tensor("wkp", [128, K], BF16, kind="ExternalInput")
    wvc_d = nc.dram_tensor("wvc", [3, K], BF16, kind="ExternalInput")
    fcw_d = nc.dram_tensor("fcw", [C, C], BF16, kind="ExternalInput")
    fcb_d = nc.dram_tensor("fcb", [C, 1], F32, kind="ExternalInput")
    msk_d = nc.dram_tensor("masks", [NSLOT, 8, 128, NBLK], BF16,
                           kind="ExternalInput")
    od_d = nc.dram_tensor("onesd", [128, 128], BF16, kind="ExternalInput")
    xres_d = nc.dram_tensor("xres", [C, NSLOT * NBLK], F32,
                            kind="ExternalInput")
    out_d = nc.dram_tensor("out", [C, NSLOT * NBLK], F32, kind="ExternalOutput")

    with tile.TileContext(nc) as tc:
        with (
            tc.tile_pool(name="wts", bufs=1) as wts,
            tc.tile_pool(name="kqv_p", bufs=1) as kqv_p,
            tc.tile_pool(name="msk_p", bufs=6) as msk_p,
            tc.tile_pool(name="ex_p", bufs=10) as ex_p,
            tc.tile_pool(name="ds_p", bufs=4) as ds_p,
            tc.tile_pool(name="f_p", bufs=2) as f_p,
            tc.tile_pool(name="o_p", bufs=2) as o_p,
            tc.tile_pool(name="tr_p", bufs=2) as tr_p,
            tc.tile_pool(name="ps_sc", bufs=4, space="PSUM") as ps_sc,
            tc.tile_pool(name="ps_d1", bufs=1, space="PSUM") as ps_d1,
            tc.tile_pool(name="ps_o", bufs=1, space="PSUM") as ps_o,
            tc.tile_pool(name="ps_fc", bufs=1, space="PSUM") as ps_fc,
        ):
            def wtile(dram, r0, rn, dt, tag):
                t = wts.tile([rn, dram.shape[1]], dt, tag=tag, name=tag)
                nc.sync.dma_start(t[:], dram[r0:r0 + rn, :])
                return t

            # phase-A-first weights (k, v) so PE can start early
            wk_t = [wtile(wk_d, 0, 128, BF16, "wk0"),
                    wtile(wk_d, 128, 128, BF16, "wk1"),
                    wtile(wkp_d, 0, 128, BF16, "wk2")]
            wv_t = [wtile(wv_d, 0, 128, BF16, "wv0"),
                    wtile(wv_d, 128, 128, BF16, "wv1"),
                    wtile(wvc_d, 0, 3, BF16, "wv2")]

            # resident x0 (+pos pad), DMAed in per-pair column chunks so
            # the first projections are not gated on the full transfer
            x0_t = [kqv_p.tile([128, N], BF16, tag=f"x0{ci}", name=f"x0{ci}")
                    for ci in range(2)]
            x0p_t = kqv_p.tile([128, N], BF16, tag="x0p", name="x0p")
            x0c_t = kqv_p.tile([3, N], BF16, tag="x0c", name="x0c")

            def chunk_dma(nbp):
                sl = slice(1024 * nbp, 1024 * (nbp + 1))
                for ci in range(2):
                    nc.sync.dma_start(x0_t[ci][:, sl],
                                      x0b_d[128 * ci:128 * (ci + 1), sl])
                nc.sync.dma_start(x0p_t[:, sl], x0p_d[:, sl])
                nc.sync.dma_start(x0c_t[:, sl], x0c_d[:, sl])

            chunk_dma(0)

            k_sb = [[None] * 8 for _ in range(2)]
            vT_sb = [None] * 32

            def emit_pair(nbp):
                for nb in (2 * nbp, 2 * nbp + 1):
                    sl = slice(NBLK * nb, NBLK * (nb + 1))
                    for kt in range(2):
                        kts = slice(128 * kt, 128 * (kt + 1))
                        pk = ps_sc.tile([128, NBLK], F32, tag="sc",
                                        name=f"pk{kt}_{nb}")
                        nc.tensor.matmul(pk[:], wk_t[0][:, kts],
                                         x0_t[0][:, sl], start=True,
                                         stop=False)
                        nc.tensor.matmul(pk[:], wk_t[1][:, kts],
                                         x0_t[1][:, sl], start=False,
                                         stop=False)
                        nc.tensor.matmul(pk[:], wk_t[2][:, kts],
                                         x0p_t[:, sl], start=False,
                                         stop=True)
                        kt_sb = kqv_p.tile([128, NBLK], BF16,
                                           tag=f"k{kt}_{nb}",
                                           name=f"k{kt}_{nb}")
                        nc.scalar.activation(kt_sb[:], pk[:], Act.Relu)
                        k_sb[kt][nb] = kt_sb
                for nb in (2 * nbp, 2 * nbp + 1):
                    for sub in range(4):
                        i = 4 * nb + sub
                        ss = slice(128 * i, 128 * (i + 1))
                        pv = ps_sc.tile([128, K], F32, tag="sc",
                                        name=f"pv{i}")
                        nc.tensor.matmul(pv[:], x0_t[0][:, ss], wv_t[0][:],
                                         start=True, stop=False)
                        nc.tensor.matmul(pv[:], x0_t[1][:, ss], wv_t[1][:],
                                         start=False, stop=False)
                        nc.tensor.matmul(pv[:], x0c_t[:, ss], wv_t[2][:],
                                         start=False, stop=True)
                        vt_sb = kqv_p.tile([128, K], BF16, tag=f"v{i}",
                                           name=f"v{i}")
                        nc.scalar.activation(vt_sb[:], pv[:], Act.Relu)
                        vT_sb[i] = vt_sb

            q_sb = [[None] * NSLOT for _ in range(2)]

            def emit_q(s):
                sl = slice(NBLK * s, NBLK * (s + 1))
                for kt in range(2):
                    kts = slice(128 * kt, 128 * (kt + 1))
                    pq = ps_sc.tile([128, NBLK], F32, tag="sc",
                                    name=f"pq{kt}_{s}")
                    nc.tensor.matmul(pq[:], wq_t[0][:, kts], xq_t[0][:, sl],
                                     start=True, stop=False)
                    nc.tensor.matmul(pq[:], wq_t[1][:, kts], xq_t[1][:, sl],
                                     start=False, stop=False)
                    nc.tensor.matmul(pq[:], wq_t[2][:, kts], xqp_t[:, sl],
                                     start=False, stop=True)
                    qt = kqv_p.tile([128, NBLK], BF16, tag=f"q{kt}_{s}",
                                    name=f"q{kt}_{s}")
                    nc.scalar.activation(qt[:], pq[:], Act.Relu)
                    q_sb[kt][s] = qt

            slot_state = {}  # s -> (po, pd)

            def emit_slot(s):
                """scores, exp, masks, den quads, attn@v for slot s."""
                M = M_S[s]
                po = [ps_o.tile([128, NBLK], F32, tag=f"o{vt}",
                                name=f"po{vt}_{s}") for vt in range(2)]
                pd = ps_d1.tile([128, NBLK], F32, tag="d1", name=f"pd{s}")
                ex_tiles = [None] * M
                for i in range(M):
                    psc = ps_sc.tile([128, NBLK], F32, tag="sc",
                                     name=f"psc{s}_{i}")
                    for kt in range(2):
                        nc.tensor.matmul(
                            psc[:],
                            k_sb[kt][i // 4][:, 128 * (i % 4):128 * (i % 4 + 1)],
                            q_sb[kt][s][:], start=(kt == 0), stop=(kt == 1))
                    ex = ex_p.tile([128, NBLK], BF16, tag="ex",
                                   name=f"ex{s}_{i}")
                    nc.scalar.activation(ex[:], psc[:], Act.Exp)
                    if i >= M - 8:
                        mk = msk_p.tile([128, NBLK], BF16, tag="mk",
                                        name=f"mk{s}_{i}")
                        nc.sync.dma_start(mk[:], msk_d[s, i - (M - 8)])
                        nc.vector.tensor_mul(ex[:], ex[:], mk[:])
                    ex_tiles[i] = ex
                    if i % 4 == 3:
                        j = i - 3
                        # attn@v for the finished quad
                        for jj in range(j, j + 4):
                            e = ex_tiles[jj]
                            for vt in range(2):
                                nc.tensor.matmul(
                                    po[vt][:],
                                    vT_sb[jj][:, 128 * vt:128 * (vt + 1)],
                                    e[:], start=(jj == 0), stop=(jj == M - 1))
                        # quad-summed denominator
                        da = ds_p.tile([128, NBLK], BF16, tag="ds",
                                       name=f"da{s}_{j}")
                        nc.vector.tensor_add(da[:], ex_tiles[j][:],
                                             ex_tiles[j + 1][:])
                        db = ds_p.tile([128, NBLK], BF16, tag="ds",
                                       name=f"db{s}_{j}")
                        nc.vector.tensor_add(db[:], ex_tiles[j + 2][:],
                                             ex_tiles[j + 3][:])
                        dsum = ds_p.tile([128, NBLK], BF16, tag="ds",
                                         name=f"ds{s}_{j}")
                        nc.vector.tensor_add(dsum[:], da[:], db[:])
                        nc.tensor.matmul(pd[:], od_t[:], dsum[:],
                                         start=(j == 0), stop=(j == M - 4))
                        for jj in range(j, j + 4):
                            ex_tiles[jj] = None
                slot_state[s] = (po, pd)

            def finalize(s):
                """normalize, fc, relu, residual, dma out for slot s."""
                po, pd = slot_state[s]
                rb = f_p.tile([128, NBLK], F32, tag="rb", name=f"rb{s}")
                nc.vector.reciprocal_approx_fast(rb[:], pd[:])
                o_sb = []
                for vt in range(2):
                    ot = o_p.tile([128, NBLK], BF16, tag=f"ob{vt}",
                                  name=f"ob{vt}_{s}")
                    nc.vector.tensor_mul(ot[:], po[vt][:], rb[:])
                    o_sb.append(ot)
                for ot in range(2):
                    pfc = ps_fc.tile([128, NBLK], F32, tag="fc",
                                     name=f"pfc{ot}_{s}")
                    for vt in range(2):
                        nc.tensor.matmul(
                            pfc[:], fcw_t[vt][:, 128 * ot:128 * (ot + 1)],
                            o_sb[vt][:], start=(vt == 0), stop=(vt == 1))
                    t_sb = tr_p.tile([128, NBLK], F32, tag=f"t{ot}",
                                     name=f"t{ot}_{s}")
                    nc.scalar.activation(t_sb[:], pfc[:], Act.Relu,
                                         bias=fcb_t[ot][:])
                    r_sb = tr_p.tile([128, NBLK], F32, tag=f"r{ot}",
                                     name=f"r{ot}_{s}")
                    nc.vector.tensor_add(
                        r_sb[:], t_sb[:],
                        xres_t[ot][:, NBLK * s:NBLK * (s + 1)])
                    nc.sync.dma_start(
                        out_d[128 * ot:128 * (ot + 1),
                              NBLK * s:NBLK * (s + 1)], r_sb[:])

            # ---- emission schedule: spread k/v pairs between slots so
            # DMA and projections overlap attention; all bf16, no dtype
            # switches anywhere.
            emit_pair(0)

            wq_t = [wtile(wq_d, 0, 128, BF16, "wq0"),
                    wtile(wq_d, 128, 128, BF16, "wq1"),
                    wtile(wqp_d, 0, 128, BF16, "wq2")]
            xq_t = [wtile(xq_d, 0, 128, BF16, "xq0"),
                    wtile(xq_d, 128, 128, BF16, "xq1")]
            xqp_t = wtile(xqp_d, 0, 128, BF16, "xqp")
            fcw_t = [wtile(fcw_d, 0, 128, BF16, "fcw0"),
                     wtile(fcw_d, 128, 128, BF16, "fcw1")]
            fcb_t = [wtile(fcb_d, 0, 128, F32, "fcb0"),
                     wtile(fcb_d, 128, 128, F32, "fcb1")]
            od_t = wtile(od_d, 0, 128, BF16, "onesd")
            xres_t = [wtile(xres_d, 128 * ot, 128, F32, f"xres{ot}")
                      for ot in range(2)]

            emit_q(0)
            emit_slot(0)
            chunk_dma(1)
            emit_pair(1)
            emit_q(1)
            emit_slot(1)
            chunk_dma(2)
            emit_pair(2)
            emit_q(2)
            finalize(0)
            emit_slot(2)
            chunk_dma(3)
            emit_pair(3)
            emit_q(3)
            finalize(1)
            emit_slot(3)
            finalize(2)
            finalize(3)

    nc.compile()
    return nc


def _host_prep(x, q_w, q_b, k_w, k_b, v_w, v_b, fc_w, fc_b):
    """Build the per-core input maps."""
    import ml_dtypes
    f32 = np.float32
    bf16 = ml_dtypes.bfloat16
    n = np.arange(N)
    px = ((n // S) / S).astype(f32)
    py = ((n % S) / S).astype(f32)
    pos3 = np.stack([px, py, np.ones(N, f32)])   # [3, N] incl bias channel

    pos_pad = np.zeros((128, N), f32)
    pos_pad[:3] = pos3

    def pad_w(w, b):
        # rows 0..1 = pos weight rows, row 2 = bias, rest zero
        p = np.zeros((128, K), f32)
        p[:2] = w.astype(f32).T[C:]
        p[2] = b.astype(f32)
        return p

    # per-role masks [NSLOT, 8, 128, 512]
    mm = np.arange(128)[:, None]
    nn = np.arange(NBLK)[None, :]
    masks = {}
    for r in range(2):
        mr = np.zeros((NSLOT, 8, 128, NBLK), f32)
        for s in range(NSLOT):
            j = BLOCKS[r][s]
            for t in range(8):
                i = M_S[s] - 8 + t
                mr[s, t] = (128 * i + mm <= 512 * j + nn)
        masks[r] = mr.astype(bf16)

    shared = {
        "wq": np.ascontiguousarray(q_w.astype(f32).T[:C]).astype(bf16),
        "wk": np.ascontiguousarray(k_w.astype(f32).T[:C]).astype(bf16),
        "wv": np.ascontiguousarray(v_w.astype(f32).T[:C]).astype(bf16),
        "wqp": pad_w(q_w, q_b).astype(bf16),
        "wkp": pad_w(k_w, k_b).astype(bf16),
        "wvc": np.ascontiguousarray(
            np.concatenate([v_w.astype(f32).T[C:],
                            v_b.astype(f32)[None, :]], 0)).astype(bf16),
        "x0p": pos_pad.astype(bf16),
        "x0c": pos3.astype(bf16),
        "fcw": np.ascontiguousarray(fc_w.astype(f32).T).astype(bf16),
        "fcb": np.ascontiguousarray(fc_b.astype(f32).reshape(C, 1)),
        "onesd": np.ones((128, 128), f32).astype(bf16),
    }

    in_maps = []
    for c in range(8):
        b, r = c // 2, c % 2
        xb = x[b].reshape(C, N).astype(f32)
        qcols = np.concatenate(
            [np.arange(NBLK * j, NBLK * (j + 1)) for j in BLOCKS[r]])
        in_maps.append(dict(
            shared,
            x0b=xb.astype(bf16),
            xq=np.ascontiguousarray(xb[:, qcols]).astype(bf16),
            xqp=np.ascontiguousarray(pos_pad[:, qcols]).astype(bf16),
            xres=np.ascontiguousarray(xb[:, qcols]),
            masks=masks[r],
        ))
    return in_maps


def _gather(results):
    out = np.empty((B, C, N), np.float32)
    for c in range(8):
        b, r = c // 2, c % 2
        oc = results[c]["out"]
        for s, j in enumerate(BLOCKS[r]):
            out[b][:, NBLK * j:NBLK * (j + 1)] = oc[:, NBLK * s:NBLK * (s + 1)]
    return out.reshape(B, C, S, S)


def run(trace=False, **inputs):
    from concourse import bass_utils
    global _PROGRAM
    if _PROGRAM is None:
        _PROGRAM = _build_program()
    in_maps = _host_prep(**inputs)
    res = bass_utils.run_bass_kernel_spmd(
        _PROGRAM, in_maps, list(range(8)), trace=trace)
    return _gather(res.results), res


def kernel(**inputs):
    out, _ = run(trace=False, **inputs)
    return out
